# revision 9
# baseline (speedup 1.0000x reference)
"""DifferentiableLengthRegulator Trainium2 kernel.

out[b,c,l] = y_mask * (sum_t x[b,c,t]*W[b,t,l]) / (sum_t W[b,t,l] + eps)
W = exp(-0.5*(l - center[b,t])^2 / (w[b,t]^2*sigma_scale^2 + eps))

Sharding: data-parallel over batch B=16 -> 8 cores x 2 batches.
Per core, per batch (banded over the frame axis; Gaussian weights vanish
outside ~5 sigma of each token chunk's centers):
  ACT : W = DerivErf(s*pos + (-s*center)) -> bf16  (per-partition scale+bias
        fold computes mu inside the ACT op: no DVE mu stage at all)
  PE  : psum[l, 0:257] += W_tc[:, lslice]^T @ [xT | ones]
  DVE : d+eps = tensor_scalar(psum cols 256); rd = 1/(d+eps);
        evac psum*rd -> bf16 (ACT takes some chunks as Copy-scale)
PSUM is one manually-rotated [CH, 8, 512] arena (8 banks): chunk j lives in
slot j%8, giving depth-8 rotation with per-range dependencies instead of
depth-2 tile-pool rotation (PE never waits on evac).
Output layout [BPC, NT, CH, GRP, C] bf16 -> 2KB contiguous DMA lines; host
reshapes to (B, C, L) fp32. x_mask / y_mask folded on host.
"""

import numpy as np
import ml_dtypes

B, C, T, L = 16, 256, 512, 4096
N_CORES = 8
BPC = B // N_CORES  # batches per core
CH = 128            # partition chunk
TCN = T // CH       # 4 token chunks
LCN = L // CH       # 32 frame chunks
GRP = 4             # frame chunks per evac group
NT = LCN // GRP     # 8 groups per batch
NSLOT = 8           # psum arena slots (banks)
EPS = 1e-8
K_DERF = 1.1283791670955126  # 2/sqrt(pi), DerivErf's constant factor
MARGIN_SIGMA = 5.0
BAND_ALIGN = 128
NWARM = 4
LOOKAHEAD = 3

_bf16 = ml_dtypes.bfloat16
_cache = {}

# DVE evac chunks per group (out of GRP=4); remainder goes to ACT as
# Copy-scale. b0's groups run while ACT also generates later W tiles.
ND_PATTERN = {0: [3] * 8, 1: [3, 2, 3, 2, 3, 2, 3, 2]}


def _center_scale(w, sigma_scale):
    """Mirror the reference's cumsum/center math (same jax backend bits)."""
    try:
        import jax.numpy as jnp

        wj = jnp.asarray(w)
        center = np.asarray(jnp.cumsum(wj, axis=1) - 0.5 * wj, dtype=np.float32)
    except Exception:
        center = (np.cumsum(w, axis=1, dtype=np.float32) - 0.5 * w).astype(np.float32)
    sigma = (w * np.float32(sigma_scale)).astype(np.float32)
    # W = DerivErf(s*mu)*sqrt(pi)/2 = exp(-(s*mu)^2), s = sqrt(0.5/(sig^2+eps))
    s = np.sqrt(np.float32(0.5) / (np.square(sigma) + np.float32(EPS))).astype(np.float32)
    return center, s


def _bands(center, w_all):
    """Per (slot, tc) aligned frame band, unioned across cores (SPMD)."""
    bands = []
    for slot in range(BPC):
        rows = center[slot::BPC]      # the 8 batches that land in this slot
        wrows = w_all[slot::BPC]
        sb = []
        for tc in range(TCN):
            seg = rows[:, tc * CH:(tc + 1) * CH]
            margin = float(MARGIN_SIGMA * wrows[:, tc * CH:(tc + 1) * CH].max() + 1.0)
            bs = max(0, int(np.floor((seg.min() - margin) / BAND_ALIGN)) * BAND_ALIGN)
            be = min(L, int(np.ceil((seg.max() + margin) / BAND_ALIGN)) * BAND_ALIGN)
            if tc == 0:
                bs = 0
            if tc == TCN - 1:
                be = L
            bs = min(bs, be - CH)
            sb.append((bs, be))
        bands.append(sb)
    for sb in bands:
        for chunk in range(LCN):
            lo = chunk * CH
            assert any(bs <= lo and lo + CH <= be for bs, be in sb), (
                f"frame chunk {chunk} uncovered; widen MARGIN_SIGMA"
            )
    return bands


def _split_excess_waits(nc, max_waits=1):
    """walrus here caps sync-waits at 1 per compute instruction; move the
    excess onto injected same-engine NoOps just before the instruction
    (waiting earlier on the same engine is always safe)."""
    from concourse import mybir

    for f in nc.m.functions:
        for blk in f.blocks:
            new = []
            for inst in blk.instructions:
                si = inst.sync_info
                if si is not None and len(si.on_wait) > max_waits:
                    waits = list(si.on_wait)
                    keep, extra = waits[-max_waits:], waits[:-max_waits]
                    for i in range(0, len(extra), max_waits):
                        nop = mybir.InstNoOp(name=f"{inst.name}-xw{i}", ins=[], outs=[])
                        nop.engine = inst.engine
                        nop.sync_info = mybir.SyncInfo(
                            on_wait=extra[i:i + max_waits], on_update=[])
                        new.append(nop)
                    inst.sync_info = mybir.SyncInfo(
                        on_wait=keep, on_update=list(si.on_update))
                new.append(inst)
            blk.instructions = new


def _slim_tile_exit(tile):
    """Drop the second all-engine barrier in Tile's exit sequence: the
    sem-clears it orders are already completed by each engine finishing its
    own instruction stream before the NEFF ends (~4us saved)."""
    if getattr(tile.TileContext, "_slim_exit", False):
        return
    ScopedClock = tile.ScopedClock

    def _drain_and_barrier(self, tick_clock, wait_clock):
        drain_inst = self.nc.sync.drain()
        wait_clock.add_sem_waits(
            drain_inst.ins, ScopedClock({None: tick_clock.global_clock}))
        self.nc.all_engine_barrier()
        popped = self.nc._tile_sem_poison_stack.pop()
        assert popped is self._sem_poison
        self.nc.clear_and_free_semaphores(list(self.sems.allocated().values()))

    tile.TileContext._drain_and_barrier = _drain_and_barrier
    tile.TileContext._slim_exit = True


def _build(band_key):
    import concourse.bass as bass
    import concourse.tile as tile
    from concourse import mybir

    _slim_tile_exit(tile)
    bands = [[(band_key[s][t][0], band_key[s][t][1]) for t in range(TCN)]
             for s in range(BPC)]
    wmax = [max(bands[s][t][1] - bands[s][t][0] for s in range(BPC))
            for t in range(TCN)]

    nc = bass.Bass("TRN2", target_bir_lowering=False, debug=False)
    # xta host layout: [b, p, tc, c] so the DMA is descriptor-light
    xta_d = nc.declare_dram_parameter("xta", [BPC, CH, TCN, C + 1], mybir.dt.bfloat16, isOutput=False)
    coefs_d = nc.declare_dram_parameter("coefs", [CH, 2 * BPC * TCN], mybir.dt.float32, isOutput=False)
    out_d = nc.declare_dram_parameter("out", [BPC, NT, CH, GRP, C], mybir.dt.bfloat16, isOutput=True)

    f32 = mybir.dt.float32
    bf16 = mybir.dt.bfloat16
    FT = mybir.ActivationFunctionType
    OP = mybir.AluOpType
    EPS_K = float(EPS) * K_DERF

    # first group (in the b0..b1 stream) whose matmuls need W(b, t)
    def first_need(b, t):
        bs = bands[b][t][0]
        return b * NT + bs // (GRP * CH)

    with tile.TileContext(nc) as tc_:
        import contextlib

        with contextlib.ExitStack() as ctx:
            consts = ctx.enter_context(tc_.tile_pool(name="consts", bufs=1))
            xta_p = ctx.enter_context(tc_.tile_pool(name="xta", bufs=2))
            w_pools = [ctx.enter_context(tc_.tile_pool(name=f"w{t}", bufs=2)) for t in range(TCN)]
            psum_p = ctx.enter_context(tc_.tile_pool(name="ps", bufs=1, space="PSUM"))
            small_p = ctx.enter_context(tc_.tile_pool(name="small", bufs=6))
            out_p = ctx.enter_context(tc_.tile_pool(name="osb", bufs=4))

            def col(tile_, idx):
                return tile_[:, idx:idx + 1]

            def cidx(q, b, t):
                # q=0 -> s (scale), q=1 -> -s*center (bias)
                return (q * BPC + b) * TCN + t

            xta_tiles = {}

            def load_xta(b, t0, t1):
                if b not in xta_tiles:
                    xta_sb = xta_p.tile([CH, TCN, C + 1], bf16, tag="xta")
                    xta_tiles[b] = xta_sb
                nc.sync.dma_start(out=xta_tiles[b][:, t0:t1, :],
                                  in_=xta_d[b, :, t0:t1, :])

            # --- startup: coefs first (it gates the first W tile, which
            # gates everything), then the warm-up's xta piece.
            coefs_sb = consts.tile([CH, 2 * BPC * TCN], f32)
            nc.sync.dma_start(out=coefs_sb, in_=coefs_d[:, :])
            load_xta(0, 0, 1)
            load_xta(0, 1, TCN)

            warm = consts.tile([CH, 1], f32)
            nc.vector.memset(warm, 0.0)
            nc.scalar.activation(out=warm, in_=warm, func=FT.Derivative_Erf)

            # pos[l] = l, built cooperatively: GpSimd iota for the head and
            # tail pieces, DVE coarse+fine composed adds for the middle
            # (GpSimd iota is 1.8ns/col; DVE is 1.04 and idle before the
            # group stream starts).
            pos_f = consts.tile([CH, L], f32)
            fine = consts.tile([CH, CH], f32)
            coarse = consts.tile([CH, LCN], f32)
            P1 = 256
            IW = max(bands[s][0][1] for s in range(BPC))
            IW2 = max(bands[s][1][1] for s in range(BPC))
            nc.gpsimd.iota(fine, pattern=[[1, CH]], base=0, channel_multiplier=0,
                           allow_small_or_imprecise_dtypes=True)
            nc.gpsimd.iota(coarse, pattern=[[CH, LCN]], base=0, channel_multiplier=0,
                           allow_small_or_imprecise_dtypes=True)
            nc.gpsimd.iota(pos_f[:, :P1], pattern=[[1, P1]], base=0,
                           channel_multiplier=0, allow_small_or_imprecise_dtypes=True)

            def pos_piece_dve(lo, hi):
                nblk = (hi - lo) // CH
                finb = bass.AP(tensor=fine.tensor, offset=fine.offset,
                               ap=[fine.ap[0], [0, nblk], [1, CH]])
                corb = bass.AP(tensor=coarse.tensor,
                               offset=coarse.offset + (lo // CH) * coarse.ap[1][0],
                               ap=[coarse.ap[0], [coarse.ap[1][0], nblk], [0, CH]])
                nc.vector.tensor_tensor(
                    out=pos_f[:, lo:hi].rearrange("p (k f) -> p k f", f=CH),
                    in0=finb, in1=corb, op=OP.add)

            pos_piece_dve(P1, IW)
            pos_piece_dve(IW, IW2)
            # tail on GpSimd, concurrent with the DVE pieces
            nc.gpsimd.iota(pos_f[:, IW2:], pattern=[[1, L - IW2]], base=IW2,
                           channel_multiplier=0, allow_small_or_imprecise_dtypes=True)

            w_tiles = {}

            def wgen(b, t, cuts=None):
                bs, be = bands[b][t]
                wt = w_pools[t].tile([CH, wmax[t]], bf16)
                edges = [bs] + [c for c in (cuts or []) if bs < c < be] + [be]
                for lo, hi in zip(edges[:-1], edges[1:]):
                    # W = 2/sqrt(pi)*exp(-(s*pos - s*center)^2); the constant
                    # cancels via rd; scale+bias fold removes the mu stage.
                    nc.scalar.activation(
                        out=wt[:, lo - bs:hi - bs], in_=pos_f[:, lo:hi],
                        func=FT.Derivative_Erf,
                        scale=col(coefs_sb, cidx(0, b, t)),
                        bias=col(coefs_sb, cidx(1, b, t)),
                    )
                w_tiles[(b, t)] = wt

            # --- psum arena: one [CH, NSLOT, 512] allocation, manual
            # rotation chunk -> slot j%NSLOT; Tile range-deps do the rest.
            arena = psum_p.tile([CH, NSLOT, 512], f32)

            def chunk_matmuls(b, g, k):
                sb = bands[b]
                j0 = (g % (NSLOT // GRP)) * GRP
                lo = (g * GRP + k) * CH
                ctc = [t for t in range(TCN) if sb[t][0] <= lo and lo + CH <= sb[t][1]]
                for i, t in enumerate(ctc):
                    off = lo - sb[t][0]
                    nc.tensor.matmul(
                        out=arena[:, j0 + k, :C + 1],
                        lhsT=w_tiles[(b, t)][:, off:off + CH],
                        rhs=xta_tiles[b][:, t, :],
                        start=(i == 0), stop=(i == len(ctc) - 1),
                    )

            def group_ops(b, g, nd):
                j0 = (g % (NSLOT // GRP)) * GRP
                for k in range(GRP):
                    chunk_matmuls(b, g, k)
                dtmp = small_p.tile([CH, GRP], f32, tag="dtmp")
                nc.vector.tensor_scalar(
                    out=dtmp, in0=arena[:, j0:j0 + GRP, C],
                    scalar1=EPS_K, scalar2=None, op0=OP.add,
                )
                rd = small_p.tile([CH, GRP], f32, tag="rd")
                nc.vector.reciprocal(out=rd, in_=dtmp)
                osb = out_p.tile([CH, GRP, C], bf16, tag="osb")
                rdb = bass.AP(tensor=rd.tensor, offset=rd.offset,
                              ap=[rd.ap[0], [rd.ap[1][0], nd], [0, C]])
                nc.vector.tensor_tensor(
                    out=osb[:, :nd, :], in0=arena[:, j0:j0 + nd, :C], in1=rdb,
                    op=OP.mult)
                for k in range(nd, GRP):
                    nc.scalar.activation(
                        out=osb[:, k, :], in_=arena[:, j0 + k, :C],
                        func=FT.Copy, scale=col(rd, k))
                nc.sync.dma_start(out=out_d[b, g], in_=osb)

            def tail_group(b, g):
                # drain: process in 2-chunk halves so half 0's evac+DMA
                # overlap half 1's matmuls; halves on independent queue rings
                j0 = (g % (NSLOT // GRP)) * GRP
                osb = out_p.tile([CH, GRP, C], bf16, tag="osb")
                for h in range(2):
                    for k in (2 * h, 2 * h + 1):
                        chunk_matmuls(b, g, k)
                    dtmp = small_p.tile([CH, 2], f32, tag="dtmp")
                    nc.vector.tensor_scalar(
                        out=dtmp, in0=arena[:, j0 + 2 * h:j0 + 2 * h + 2, C],
                        scalar1=EPS_K, scalar2=None, op0=OP.add,
                    )
                    rd = small_p.tile([CH, 2], f32, tag="rd")
                    nc.vector.reciprocal(out=rd, in_=dtmp)
                    rdb = bass.AP(tensor=rd.tensor, offset=rd.offset,
                                  ap=[rd.ap[0], [rd.ap[1][0], 1], [0, C]])
                    nc.vector.tensor_tensor(
                        out=osb[:, 2 * h, :], in0=arena[:, j0 + 2 * h, :C],
                        in1=rdb, op=OP.mult)
                    nc.scalar.activation(
                        out=osb[:, 2 * h + 1, :], in_=arena[:, j0 + 2 * h + 1, :C],
                        func=FT.Copy, scale=col(rd, 1))
                    eng = nc.sync if h == 0 else nc.scalar
                    eng.dma_start(out=out_d[b, g, :, 2 * h:2 * h + 2, :],
                                  in_=osb[:, 2 * h:2 * h + 2, :])

            # --- schedule. Emission order per engine = execution order.
            # W tiles are emitted just-in-time, LOOKAHEAD groups before first
            # use; groups whose ACT copies would queue behind a DERF run
            # their whole evac on the DVE (nd=GRP).
            stream = [(b, g) for b in range(BPC) for g in range(NT)]
            need = sorted(
                ((first_need(b, t), b, t) for b in range(BPC) for t in range(TCN)),
            )
            emitted = set()

            def wgens_due(pos_idx):
                due = []
                for fn, b, t in need:
                    if fn <= pos_idx and (b, t) not in emitted:
                        emitted.add((b, t))
                        due.append((b, t))
                return due

            plan = []        # ('w', b, t) | ('g', b, g)
            for (b, t) in wgens_due(1):
                plan.append(('w', b, t))
            for idx, (b, g) in enumerate(stream):
                plan.append(('g', b, g))
                for (b2, t2) in wgens_due(idx + LOOKAHEAD):
                    plan.append(('w', b2, t2))

            for kind, x, y in plan:
                if kind == 'w':
                    wgen(x, y, cuts=[CH, IW] if (x, y) == (0, 0) else None)
                    if (x, y) == (0, 0):
                        # PE warm-up on real data bridges the pre-stream gap
                        for _ in range(NWARM):
                            nc.tensor.matmul(
                                out=arena[:, NSLOT - 1, :C + 1],
                                lhsT=w_tiles[(0, 0)][:, :CH],
                                rhs=xta_tiles[0][:, 0, :], start=True, stop=True,
                            )
                        load_xta(1, 0, TCN)
                else:
                    idx = plan.index((kind, x, y))
                    if (x, y) in (stream[-1], stream[-2]):
                        tail_group(x, y)
                    else:
                        nxt = plan[idx + 1:idx + 2]
                        nd = GRP if (nxt and nxt[0][0] == 'w') else ND_PATTERN[x][y]
                        group_ops(x, y, nd)
    return nc


def _prepare_inputs(x, w, x_mask, y_mask, sigma_scale):
    center, s = _center_scale(w, sigma_scale[0])
    bands = _bands(center, w)
    nb = (-(s * center)).astype(np.float32)    # bias: -s*center (one f32 round)

    xm = np.broadcast_to(x_mask.reshape(B, T), (B, T)).astype(np.float32)
    if not np.all(xm == 1.0):
        x = (x * xm[:, None, :]).astype(np.float32)

    xt = np.ascontiguousarray(x.transpose(0, 2, 1))          # (B, T, C)
    xta = np.concatenate([xt, np.ones((B, T, 1), np.float32)], axis=2)
    # device layout [b, p, tc, c] for a descriptor-light DMA
    xta = np.ascontiguousarray(
        xta.reshape(B, TCN, CH, C + 1).transpose(0, 2, 1, 3)).astype(_bf16)

    in_maps = []
    for core in range(N_CORES):
        bsel = [core * BPC + s_ for s_ in range(BPC)]
        coefs = np.empty((2, BPC, TCN, CH), np.float32)
        for s_, bb in enumerate(bsel):
            coefs[0, s_] = s[bb].reshape(TCN, CH)
            coefs[1, s_] = nb[bb].reshape(TCN, CH)
        in_maps.append({
            "xta": xta[bsel],
            "coefs": np.ascontiguousarray(
                coefs.reshape(2 * BPC * TCN, CH).T),          # [CH, 16]
        })
    band_key = tuple(tuple(tuple(p) for p in sb) for sb in bands)
    return in_maps, band_key


def kernel(x, w, x_mask, y_mask, sigma_scale):
    x = np.asarray(x, dtype=np.float32)
    w = np.asarray(w, dtype=np.float32)
    x_mask = np.asarray(x_mask, dtype=np.float32)
    y_mask = np.asarray(y_mask, dtype=np.float32)
    sigma_scale = np.asarray(sigma_scale, dtype=np.float32)
    assert x.shape == (B, C, T) and w.shape == (B, T)

    in_maps, band_key = _prepare_inputs(x, w, x_mask, y_mask, sigma_scale)

    if band_key not in _cache:
        nc = _build(band_key)
        _split_excess_waits(nc)
        _cache[band_key] = nc
    nc = _cache[band_key]

    from concourse.bass_utils import run_bass_kernel_spmd

    res = run_bass_kernel_spmd(nc, in_maps, list(range(N_CORES)), trace=False)
    outs = []
    for i in range(N_CORES):
        o = np.asarray(res.results[i]["out"])                # (BPC, NT, CH, GRP, C)
        o = o.astype(np.float32).transpose(0, 1, 3, 2, 4).reshape(BPC, L, C)
        outs.append(o)
    full = np.concatenate(outs, axis=0).transpose(0, 2, 1)   # (B, C, L)
    ym = np.broadcast_to(y_mask.reshape(B, L), (B, L)).astype(np.float32)
    if not np.all(ym == 1.0):
        full = full * ym[:, None, :]
    return full


# revision 12
# speedup vs baseline: 1.0063x; 1.0063x over previous
"""DifferentiableLengthRegulator Trainium2 kernel.

out[b,c,l] = y_mask * (sum_t x[b,c,t]*W[b,t,l]) / (sum_t W[b,t,l] + eps)
W = exp(-0.5*(l - center[b,t])^2 / (w[b,t]^2*sigma_scale^2 + eps))

Sharding: data-parallel over batch B=16 -> 8 cores x 2 batches.
Per core, per batch (banded over the frame axis; Gaussian weights vanish
outside ~5 sigma of each token chunk's centers):
  ACT : W = DerivErf(s*pos + (-s*center)) -> bf16  (per-partition scale+bias
        fold computes mu inside the ACT op: no DVE mu stage at all)
  PE  : psum[l, 0:257] += W_tc[:, lslice]^T @ [xT | ones]
  DVE : d+eps = tensor_scalar(psum cols 256); rd = 1/(d+eps);
        evac psum*rd -> bf16 (ACT takes some chunks as Copy-scale)
PSUM is one manually-rotated [CH, 8, 512] arena (8 banks): chunk j lives in
slot j%8, giving depth-8 rotation with per-range dependencies instead of
depth-2 tile-pool rotation (PE never waits on evac).
Output layout [BPC, NT, CH, GRP, C] bf16 -> 2KB contiguous DMA lines; host
reshapes to (B, C, L) fp32. x_mask / y_mask folded on host.
"""

import numpy as np
import ml_dtypes

B, C, T, L = 16, 256, 512, 4096
N_CORES = 8
BPC = B // N_CORES  # batches per core
CH = 128            # partition chunk
TCN = T // CH       # 4 token chunks
LCN = L // CH       # 32 frame chunks
GRP = 4             # frame chunks per evac group
NT = LCN // GRP     # 8 groups per batch
NSLOT = 8           # psum arena slots (banks)
EPS = 1e-8
K_DERF = 1.1283791670955126  # 2/sqrt(pi), DerivErf's constant factor
MARGIN_SIGMA = 5.0
BAND_ALIGN = 128
NWARM = 4
LOOKAHEAD = 3

_bf16 = ml_dtypes.bfloat16
_cache = {}

# DVE evac chunks per group (out of GRP=4); remainder goes to ACT as
# Copy-scale. b0's groups run while ACT also generates later W tiles.
ND_PATTERN = {0: [3] * 8, 1: [3, 2, 3, 2, 3, 2, 3, 2]}


def _center_scale(w, sigma_scale):
    """Mirror the reference's cumsum/center math (same jax backend bits)."""
    try:
        import jax.numpy as jnp

        wj = jnp.asarray(w)
        center = np.asarray(jnp.cumsum(wj, axis=1) - 0.5 * wj, dtype=np.float32)
    except Exception:
        center = (np.cumsum(w, axis=1, dtype=np.float32) - 0.5 * w).astype(np.float32)
    sigma = (w * np.float32(sigma_scale)).astype(np.float32)
    # W = DerivErf(s*mu)*sqrt(pi)/2 = exp(-(s*mu)^2), s = sqrt(0.5/(sig^2+eps))
    s = np.sqrt(np.float32(0.5) / (np.square(sigma) + np.float32(EPS))).astype(np.float32)
    return center, s


def _bands(center, w_all):
    """Per (slot, tc) aligned frame band, unioned across cores (SPMD)."""
    bands = []
    for slot in range(BPC):
        rows = center[slot::BPC]      # the 8 batches that land in this slot
        wrows = w_all[slot::BPC]
        sb = []
        for tc in range(TCN):
            seg = rows[:, tc * CH:(tc + 1) * CH]
            margin = float(MARGIN_SIGMA * wrows[:, tc * CH:(tc + 1) * CH].max() + 1.0)
            bs = max(0, int(np.floor((seg.min() - margin) / BAND_ALIGN)) * BAND_ALIGN)
            be = min(L, int(np.ceil((seg.max() + margin) / BAND_ALIGN)) * BAND_ALIGN)
            if tc == 0:
                bs = 0
            if tc == TCN - 1:
                be = L
            bs = min(bs, be - CH)
            sb.append((bs, be))
        bands.append(sb)
    for sb in bands:
        for chunk in range(LCN):
            lo = chunk * CH
            assert any(bs <= lo and lo + CH <= be for bs, be in sb), (
                f"frame chunk {chunk} uncovered; widen MARGIN_SIGMA"
            )
    return bands


def _split_excess_waits(nc, max_waits=1):
    """walrus here caps sync-waits at 1 per compute instruction; move the
    excess onto injected same-engine NoOps just before the instruction
    (waiting earlier on the same engine is always safe)."""
    from concourse import mybir

    for f in nc.m.functions:
        for blk in f.blocks:
            new = []
            for inst in blk.instructions:
                si = inst.sync_info
                if si is not None and len(si.on_wait) > max_waits:
                    waits = list(si.on_wait)
                    keep, extra = waits[-max_waits:], waits[:-max_waits]
                    for i in range(0, len(extra), max_waits):
                        nop = mybir.InstNoOp(name=f"{inst.name}-xw{i}", ins=[], outs=[])
                        nop.engine = inst.engine
                        nop.sync_info = mybir.SyncInfo(
                            on_wait=extra[i:i + max_waits], on_update=[])
                        new.append(nop)
                    inst.sync_info = mybir.SyncInfo(
                        on_wait=keep, on_update=list(si.on_update))
                new.append(inst)
            blk.instructions = new


def _slim_tile_exit(tile):
    """Drop the second all-engine barrier in Tile's exit sequence: the
    sem-clears it orders are already completed by each engine finishing its
    own instruction stream before the NEFF ends (~4us saved)."""
    if getattr(tile.TileContext, "_slim_exit", False):
        return
    ScopedClock = tile.ScopedClock

    def _drain_and_barrier(self, tick_clock, wait_clock):
        drain_inst = self.nc.sync.drain()
        wait_clock.add_sem_waits(
            drain_inst.ins, ScopedClock({None: tick_clock.global_clock}))
        self.nc.all_engine_barrier()
        popped = self.nc._tile_sem_poison_stack.pop()
        assert popped is self._sem_poison
        self.nc.clear_and_free_semaphores(list(self.sems.allocated().values()))

    tile.TileContext._drain_and_barrier = _drain_and_barrier
    tile.TileContext._slim_exit = True


def _build(band_key):
    import concourse.bass as bass
    import concourse.tile as tile
    from concourse import mybir

    _slim_tile_exit(tile)
    bands = [[(band_key[s][t][0], band_key[s][t][1]) for t in range(TCN)]
             for s in range(BPC)]
    wmax = [max(bands[s][t][1] - bands[s][t][0] for s in range(BPC))
            for t in range(TCN)]

    nc = bass.Bass("TRN2", target_bir_lowering=False, debug=False)
    # xta host layout: [b, p, tc, c] so the DMA is descriptor-light
    xta_d = nc.declare_dram_parameter("xta", [BPC, CH, TCN, C + 1], mybir.dt.bfloat16, isOutput=False)
    coefs_d = nc.declare_dram_parameter("coefs", [CH, 2 * BPC * TCN], mybir.dt.float32, isOutput=False)
    out_d = nc.declare_dram_parameter("out", [BPC, NT, CH, GRP, C], mybir.dt.bfloat16, isOutput=True)

    f32 = mybir.dt.float32
    bf16 = mybir.dt.bfloat16
    FT = mybir.ActivationFunctionType
    OP = mybir.AluOpType
    EPS_K = float(EPS) * K_DERF

    # first group (in the b0..b1 stream) whose matmuls need W(b, t)
    def first_need(b, t):
        bs = bands[b][t][0]
        return b * NT + bs // (GRP * CH)

    with tile.TileContext(nc) as tc_:
        import contextlib

        with contextlib.ExitStack() as ctx:
            consts = ctx.enter_context(tc_.tile_pool(name="consts", bufs=1))
            xta_p = ctx.enter_context(tc_.tile_pool(name="xta", bufs=2))
            w_pools = [ctx.enter_context(tc_.tile_pool(name=f"w{t}", bufs=2)) for t in range(TCN)]
            psum_p = ctx.enter_context(tc_.tile_pool(name="ps", bufs=1, space="PSUM"))
            small_p = ctx.enter_context(tc_.tile_pool(name="small", bufs=6))
            out_p = ctx.enter_context(tc_.tile_pool(name="osb", bufs=4))

            def col(tile_, idx):
                return tile_[:, idx:idx + 1]

            def cidx(q, b, t):
                # q=0 -> s (scale), q=1 -> -s*center (bias)
                return (q * BPC + b) * TCN + t

            xta_tiles = {}

            def load_xta(b, t0, t1):
                if b not in xta_tiles:
                    xta_sb = xta_p.tile([CH, TCN, C + 1], bf16, tag="xta")
                    xta_tiles[b] = xta_sb
                nc.sync.dma_start(out=xta_tiles[b][:, t0:t1, :],
                                  in_=xta_d[b, :, t0:t1, :])

            # --- startup: coefs first (it gates the first W tile, which
            # gates everything), then the warm-up's xta piece.
            coefs_sb = consts.tile([CH, 2 * BPC * TCN], f32)
            nc.sync.dma_start(out=coefs_sb, in_=coefs_d[:, :])
            load_xta(0, 0, 1)
            load_xta(0, 1, TCN)

            warm = consts.tile([CH, 1], f32)
            nc.vector.memset(warm, 0.0)
            nc.scalar.activation(out=warm, in_=warm, func=FT.Derivative_Erf)

            # pos[l] = l, built cooperatively: GpSimd iota for the head and
            # tail pieces, DVE coarse+fine composed adds for the middle
            # (GpSimd iota is 1.8ns/col; DVE is 1.04 and idle before the
            # group stream starts).
            pos_f = consts.tile([CH, L], f32)
            fine = consts.tile([CH, CH], f32)
            coarse = consts.tile([CH, LCN], f32)
            P1 = 256
            IW = max(bands[s][0][1] for s in range(BPC))
            IW2 = max(bands[s][1][1] for s in range(BPC))
            nc.gpsimd.iota(fine, pattern=[[1, CH]], base=0, channel_multiplier=0,
                           allow_small_or_imprecise_dtypes=True)
            nc.gpsimd.iota(coarse, pattern=[[CH, LCN]], base=0, channel_multiplier=0,
                           allow_small_or_imprecise_dtypes=True)
            nc.gpsimd.iota(pos_f[:, :P1], pattern=[[1, P1]], base=0,
                           channel_multiplier=0, allow_small_or_imprecise_dtypes=True)

            def pos_piece_dve(lo, hi):
                nblk = (hi - lo) // CH
                finb = bass.AP(tensor=fine.tensor, offset=fine.offset,
                               ap=[fine.ap[0], [0, nblk], [1, CH]])
                corb = bass.AP(tensor=coarse.tensor,
                               offset=coarse.offset + (lo // CH) * coarse.ap[1][0],
                               ap=[coarse.ap[0], [coarse.ap[1][0], nblk], [0, CH]])
                nc.vector.tensor_tensor(
                    out=pos_f[:, lo:hi].rearrange("p (k f) -> p k f", f=CH),
                    in0=finb, in1=corb, op=OP.add)

            pos_piece_dve(P1, IW)
            # remaining pieces on GpSimd, concurrent with the DVE piece;
            # split so each W tile's band completes in time for its first use
            M1 = (IW2 + 896) // CH * CH
            for lo, hi in ((IW, IW2), (IW2, M1), (M1, L)):
                if hi > lo:
                    nc.gpsimd.iota(pos_f[:, lo:hi], pattern=[[1, hi - lo]], base=lo,
                                   channel_multiplier=0,
                                   allow_small_or_imprecise_dtypes=True)

            w_tiles = {}

            def wgen(b, t, cuts=None):
                bs, be = bands[b][t]
                wt = w_pools[t].tile([CH, wmax[t]], bf16)
                edges = [bs] + [c for c in (cuts or []) if bs < c < be] + [be]
                for lo, hi in zip(edges[:-1], edges[1:]):
                    # W = 2/sqrt(pi)*exp(-(s*pos - s*center)^2); the constant
                    # cancels via rd; scale+bias fold removes the mu stage.
                    nc.scalar.activation(
                        out=wt[:, lo - bs:hi - bs], in_=pos_f[:, lo:hi],
                        func=FT.Derivative_Erf,
                        scale=col(coefs_sb, cidx(0, b, t)),
                        bias=col(coefs_sb, cidx(1, b, t)),
                    )
                w_tiles[(b, t)] = wt

            # --- psum arena: one [CH, NSLOT, 512] allocation, manual
            # rotation chunk -> slot j%NSLOT; Tile range-deps do the rest.
            arena = psum_p.tile([CH, NSLOT, 512], f32)

            def chunk_matmuls(b, g, k):
                sb = bands[b]
                j0 = (g % (NSLOT // GRP)) * GRP
                lo = (g * GRP + k) * CH
                ctc = [t for t in range(TCN) if sb[t][0] <= lo and lo + CH <= sb[t][1]]
                for i, t in enumerate(ctc):
                    off = lo - sb[t][0]
                    nc.tensor.matmul(
                        out=arena[:, j0 + k, :C + 1],
                        lhsT=w_tiles[(b, t)][:, off:off + CH],
                        rhs=xta_tiles[b][:, t, :],
                        start=(i == 0), stop=(i == len(ctc) - 1),
                    )

            def group_ops(b, g, nd):
                j0 = (g % (NSLOT // GRP)) * GRP
                for k in range(GRP):
                    chunk_matmuls(b, g, k)
                dtmp = small_p.tile([CH, GRP], f32, tag="dtmp")
                nc.vector.tensor_scalar(
                    out=dtmp, in0=arena[:, j0:j0 + GRP, C],
                    scalar1=EPS_K, scalar2=None, op0=OP.add,
                )
                rd = small_p.tile([CH, GRP], f32, tag="rd")
                nc.vector.reciprocal(out=rd, in_=dtmp)
                osb = out_p.tile([CH, GRP, C], bf16, tag="osb")
                rdb = bass.AP(tensor=rd.tensor, offset=rd.offset,
                              ap=[rd.ap[0], [rd.ap[1][0], nd], [0, C]])
                nc.vector.tensor_tensor(
                    out=osb[:, :nd, :], in0=arena[:, j0:j0 + nd, :C], in1=rdb,
                    op=OP.mult)
                for k in range(nd, GRP):
                    nc.scalar.activation(
                        out=osb[:, k, :], in_=arena[:, j0 + k, :C],
                        func=FT.Copy, scale=col(rd, k))
                # late groups go on the (otherwise idle) scalar ring so the
                # sync ring's descriptor backlog is drained before the tail
                eng = nc.scalar if g in (NT - 4, NT - 3) and b == BPC - 1 else nc.sync
                eng.dma_start(out=out_d[b, g], in_=osb)

            def tail_group(b, g, last):
                # drain: process in 2-chunk halves so half 0's evac+DMA
                # overlap half 1's matmuls; the final chunk takes the
                # shortest chain (DVE TT -> sync DMA on a drained ring)
                j0 = (g % (NSLOT // GRP)) * GRP
                osb = out_p.tile([CH, GRP, C], bf16, tag="osb")
                for h in range(2):
                    for k in (2 * h, 2 * h + 1):
                        chunk_matmuls(b, g, k)
                    dtmp = small_p.tile([CH, 2], f32, tag="dtmp")
                    nc.vector.tensor_scalar(
                        out=dtmp, in0=arena[:, j0 + 2 * h:j0 + 2 * h + 2, C],
                        scalar1=EPS_K, scalar2=None, op0=OP.add,
                    )
                    rd = small_p.tile([CH, 2], f32, tag="rd")
                    nc.vector.reciprocal(out=rd, in_=dtmp)
                    rdb = bass.AP(tensor=rd.tensor, offset=rd.offset,
                                  ap=[rd.ap[0], [rd.ap[1][0], 1], [0, C]])
                    if last and h == 1:
                        # c30: ACT copy -> scalar ring; c31: DVE TT -> sync
                        nc.scalar.activation(
                            out=osb[:, 2, :], in_=arena[:, j0 + 2, :C],
                            func=FT.Copy, scale=col(rd, 0))
                        nc.scalar.dma_start(out=out_d[b, g, :, 2:3, :],
                                            in_=osb[:, 2:3, :])
                        rdb1 = bass.AP(tensor=rd.tensor,
                                       offset=rd.offset + rd.ap[1][0],
                                       ap=[rd.ap[0], [rd.ap[1][0], 1], [0, C]])
                        nc.vector.tensor_tensor(
                            out=osb[:, 3, :], in0=arena[:, j0 + 3, :C],
                            in1=rdb1, op=OP.mult)
                        nc.sync.dma_start(out=out_d[b, g, :, 3:4, :],
                                          in_=osb[:, 3:4, :])
                        return
                    nc.vector.tensor_tensor(
                        out=osb[:, 2 * h, :], in0=arena[:, j0 + 2 * h, :C],
                        in1=rdb, op=OP.mult)
                    nc.scalar.activation(
                        out=osb[:, 2 * h + 1, :], in_=arena[:, j0 + 2 * h + 1, :C],
                        func=FT.Copy, scale=col(rd, 1))
                    eng = nc.scalar if h == 0 else nc.sync
                    eng.dma_start(out=out_d[b, g, :, 2 * h:2 * h + 2, :],
                                  in_=osb[:, 2 * h:2 * h + 2, :])

            # --- schedule. Emission order per engine = execution order.
            # W tiles are emitted just-in-time, LOOKAHEAD groups before first
            # use; groups whose ACT copies would queue behind a DERF run
            # their whole evac on the DVE (nd=GRP).
            stream = [(b, g) for b in range(BPC) for g in range(NT)]
            need = sorted(
                ((first_need(b, t), b, t) for b in range(BPC) for t in range(TCN)),
            )
            emitted = set()

            def wgens_due(pos_idx):
                due = []
                for fn, b, t in need:
                    if fn <= pos_idx and (b, t) not in emitted:
                        emitted.add((b, t))
                        due.append((b, t))
                return due

            plan = []        # ('w', b, t) | ('g', b, g)
            for (b, t) in wgens_due(1):
                plan.append(('w', b, t))
            for idx, (b, g) in enumerate(stream):
                plan.append(('g', b, g))
                for (b2, t2) in wgens_due(idx + LOOKAHEAD):
                    plan.append(('w', b2, t2))

            for kind, x, y in plan:
                if kind == 'w':
                    wgen(x, y, cuts=[CH, IW] if (x, y) == (0, 0) else None)
                    if (x, y) == (0, 0):
                        # PE warm-up on real data bridges the pre-stream gap
                        for _ in range(NWARM):
                            nc.tensor.matmul(
                                out=arena[:, NSLOT - 1, :C + 1],
                                lhsT=w_tiles[(0, 0)][:, :CH],
                                rhs=xta_tiles[0][:, 0, :], start=True, stop=True,
                            )
                        load_xta(1, 0, TCN)
                else:
                    idx = plan.index((kind, x, y))
                    if (x, y) in (stream[-1], stream[-2]):
                        tail_group(x, y, last=(x, y) == stream[-1])
                    else:
                        nxt = plan[idx + 1:idx + 2]
                        nd = GRP if (nxt and nxt[0][0] == 'w') else ND_PATTERN[x][y]
                        group_ops(x, y, nd)
    return nc


def _prepare_inputs(x, w, x_mask, y_mask, sigma_scale):
    center, s = _center_scale(w, sigma_scale[0])
    bands = _bands(center, w)
    nb = (-(s * center)).astype(np.float32)    # bias: -s*center (one f32 round)

    xm = np.broadcast_to(x_mask.reshape(B, T), (B, T)).astype(np.float32)
    if not np.all(xm == 1.0):
        x = (x * xm[:, None, :]).astype(np.float32)

    xt = np.ascontiguousarray(x.transpose(0, 2, 1))          # (B, T, C)
    xta = np.concatenate([xt, np.ones((B, T, 1), np.float32)], axis=2)
    # device layout [b, p, tc, c] for a descriptor-light DMA
    xta = np.ascontiguousarray(
        xta.reshape(B, TCN, CH, C + 1).transpose(0, 2, 1, 3)).astype(_bf16)

    in_maps = []
    for core in range(N_CORES):
        bsel = [core * BPC + s_ for s_ in range(BPC)]
        coefs = np.empty((2, BPC, TCN, CH), np.float32)
        for s_, bb in enumerate(bsel):
            coefs[0, s_] = s[bb].reshape(TCN, CH)
            coefs[1, s_] = nb[bb].reshape(TCN, CH)
        in_maps.append({
            "xta": xta[bsel],
            "coefs": np.ascontiguousarray(
                coefs.reshape(2 * BPC * TCN, CH).T),          # [CH, 16]
        })
    band_key = tuple(tuple(tuple(p) for p in sb) for sb in bands)
    return in_maps, band_key


def kernel(x, w, x_mask, y_mask, sigma_scale):
    x = np.asarray(x, dtype=np.float32)
    w = np.asarray(w, dtype=np.float32)
    x_mask = np.asarray(x_mask, dtype=np.float32)
    y_mask = np.asarray(y_mask, dtype=np.float32)
    sigma_scale = np.asarray(sigma_scale, dtype=np.float32)
    assert x.shape == (B, C, T) and w.shape == (B, T)

    in_maps, band_key = _prepare_inputs(x, w, x_mask, y_mask, sigma_scale)

    if band_key not in _cache:
        nc = _build(band_key)
        _split_excess_waits(nc)
        _cache[band_key] = nc
    nc = _cache[band_key]

    from concourse.bass_utils import run_bass_kernel_spmd

    res = run_bass_kernel_spmd(nc, in_maps, list(range(N_CORES)), trace=False)
    outs = []
    for i in range(N_CORES):
        o = np.asarray(res.results[i]["out"])                # (BPC, NT, CH, GRP, C)
        o = o.astype(np.float32).transpose(0, 1, 3, 2, 4).reshape(BPC, L, C)
        outs.append(o)
    full = np.concatenate(outs, axis=0).transpose(0, 2, 1)   # (B, C, L)
    ym = np.broadcast_to(y_mask.reshape(B, L), (B, L)).astype(np.float32)
    if not np.all(ym == 1.0):
        full = full * ym[:, None, :]
    return full
